# revision 2
# baseline (speedup 1.0000x reference)
"""RNN-T Joiner kernel for Trainium2, data-parallel over batch N across 8 NeuronCores.

Per core (one batch element):
  enc_T[J,T] = enc_W @ x_enc.T          (projection, bf16 matmul, fp32 accum, stays in PSUM)
  dec_T[J,U] = dec_W @ x_dec.T + (enc_b+dec_b)
  act[J,(u,t)] = tanh(enc_T[:,t] + dec_T[:,u])   (ScalarE reads enc from PSUM, dec as bias)
  out[t,u,:] = act.T @ out_W.T + out_b           (PE, act stationary, fp32 PSUM)

Scheduling notes (from trace analysis):
- each dma_start costs ~620ns of serialized issue time on its sequencer; TRN2 has
  two HWDGE rings (sync + scalar), so input/output DMA issues are split across both.
- output rows for one t are contiguous across u in HBM, so staging C consecutive
  u-iterations in SBUF lets one descriptor cover C*2000 bytes; group sizes taper
  ([2,2,4,8*6,4,2,1,1]) so stores start early and the post-matmul drain is ~1 u.
"""

import sys

import numpy as np

try:
    import concourse.bass as bass
except ImportError:
    sys.path.insert(0, "/opt/trn_rl_repo")
    import concourse.bass as bass

import ml_dtypes

import concourse.mybir as mybir
import concourse.tile as tile
from concourse import bacc
from concourse.bass import ds, ts
from concourse.bass_utils import run_bass_kernel_spmd

N, T, U = 8, 512, 64
E = D = J = 512
V = 500
P = 128
JC = J // P  # 4 chunks of J on partitions
TB = T // P  # 4 blocks of T rows per output tile
F32 = mybir.dt.float32
BF16 = mybir.dt.bfloat16

NUM_CORES = 8

# u-iterations per output staging group: start small so stores begin early,
# 8 in steady state (16KB contiguous per descriptor), taper so the final
# drain after the last matmul is a single u (1MB across 4 DMAs).
GROUPS = [2, 2, 4] + [8] * 6 + [4, 2, 1, 1]
assert sum(GROUPS) == U
CMAX = max(GROUPS)


def build_nc() -> bass.Bass:
    nc = bacc.Bacc(
        "TRN2", target_bir_lowering=False, debug=False, num_devices=NUM_CORES
    )
    xT = nc.declare_dram_parameter("xT", [E, T], BF16, isOutput=False)
    dT = nc.declare_dram_parameter("dT", [D, U], BF16, isOutput=False)
    WeT = nc.declare_dram_parameter("WeT", [E, J], BF16, isOutput=False)
    WdT = nc.declare_dram_parameter("WdT", [D, J], BF16, isOutput=False)
    WoT = nc.declare_dram_parameter("WoT", [J, V], BF16, isOutput=False)
    cb = nc.declare_dram_parameter("cb", [J], F32, isOutput=False)
    ob = nc.declare_dram_parameter("ob", [P, V], F32, isOutput=False)
    out = nc.declare_dram_parameter("out", [T, U, V], F32, isOutput=True)

    with tile.TileContext(nc) as tc:
        with (
            tc.tile_pool(name="const", bufs=1) as const_pool,
            tc.tile_pool(name="acts", bufs=3) as act_pool,
            tc.tile_pool(name="obig", bufs=2) as obig_pool,
            tc.tile_pool(name="psenc", bufs=4, space="PSUM") as enc_psum_pool,
            tc.tile_pool(name="psum", bufs=4, space="PSUM") as psum_pool,
        ):
            # ---- persistent SBUF tensors -------------------------------------
            # chunked layouts: [P, chunk, free]
            xT_sb = const_pool.tile([P, E // P, T], BF16, tag="xT")
            dT_sb = const_pool.tile([P, D // P, U], BF16, tag="dT")
            WeT_sb = const_pool.tile([P, E // P, J], BF16, tag="WeT")
            WdT_sb = const_pool.tile([P, D // P, J], BF16, tag="WdT")
            Wo_sb = const_pool.tile([P, JC, V], BF16, tag="WoT")
            cb_sb = const_pool.tile([P, JC], F32, tag="cb")
            ob_sb = const_pool.tile([P, V], F32, tag="ob")
            dec_sb = const_pool.tile([P, JC, U], F32, tag="decT")
            warm_sb = const_pool.tile([P, 64], BF16, tag="warm")

            nc.vector.memset(warm_sb[:], 0.0)

            # ---- input DMAs: coarse transfers, issue split across the two
            # HWDGE rings (sync + scalar); each issue serializes ~620ns on its
            # sequencer, so order = consumption order.
            xT_r = xT.rearrange("(c p) t -> p c t", p=P)
            WeT_r = WeT.rearrange("(c p) j -> p c j", p=P)
            Wo_r = WoT.rearrange("(c p) v -> p c v", p=P)
            # sync ring: enc-projection operands (first chunk alone so the
            # first matmuls can start as early as possible)
            nc.sync.dma_start(xT_sb[:, 0, :], xT_r[:, 0, :])
            nc.sync.dma_start(WeT_sb[:, 0, :], WeT_r[:, 0, :])
            nc.sync.dma_start(xT_sb[:, 1:4, :], xT_r[:, 1:4, :])
            nc.sync.dma_start(WeT_sb[:, 1:4, :], WeT_r[:, 1:4, :])
            # scalar ring: dec-projection operands + output weights/biases
            nc.scalar.dma_start(WdT_sb[:], WdT.rearrange("(c p) j -> p c j", p=P))
            nc.scalar.dma_start(dT_sb[:], dT.rearrange("(c p) u -> p c u", p=P))
            nc.scalar.dma_start(Wo_sb[:, 0:2, :], Wo_r[:, 0:2, :])
            nc.scalar.dma_start(cb_sb[:], cb.rearrange("(c p) -> p c", p=P))
            nc.scalar.dma_start(Wo_sb[:, 2:4, :], Wo_r[:, 2:4, :])
            nc.scalar.dma_start(ob_sb[:], ob[:])

            # ---- projections -------------------------------------------------
            # enc stays in PSUM for the whole kernel (4 banks pinned); tanh
            # reads it directly with dec as the per-partition bias.
            ps_enc = [
                enc_psum_pool.tile([P, T], F32, tag="pse", name=f"ps_enc_{jm}")
                for jm in range(JC)
            ]
            ps_dec = [
                psum_pool.tile([P, T], F32, tag="ps", name=f"ps_dec_{jm}")
                for jm in range(JC)
            ]

            # PE warm-up: dummy matmuls while the input DMAs land, so the HAM
            # clock-gate lifts (1.2 -> 2.4 GHz) and the PE is busy until the
            # first projection operands arrive. enc's start=True resets the bank.
            for w in range(44):
                nc.tensor.matmul(
                    ps_enc[0][:64, :64],
                    lhsT=warm_sb[:, :64],
                    rhs=warm_sb[:],
                    start=True,
                    stop=True,
                    skip_group_check=True,
                )

            # ek outer so the first matmuls only need chunk 0 of the DMAs;
            # enc and dec interleaved per chunk so each DMA round feeds both.
            for ek in range(E // P):
                for jm in range(JC):
                    nc.tensor.matmul(
                        ps_enc[jm][:],
                        lhsT=WeT_sb[:, ek, ts(jm, P)],
                        rhs=xT_sb[:, ek, :],
                        start=(ek == 0),
                        stop=(ek == E // P - 1),
                        skip_group_check=True,
                    )
                for jm in range(JC):
                    nc.tensor.matmul(
                        ps_dec[jm][:, :U],
                        lhsT=WdT_sb[:, ek, ts(jm, P)],
                        rhs=dT_sb[:, ek, :],
                        start=(ek == 0),
                        stop=(ek == E // P - 1),
                        skip_group_check=True,
                    )
            for jm in range(JC):
                nc.scalar.activation(
                    dec_sb[:, jm, :],
                    ps_dec[jm][:, :U],
                    mybir.ActivationFunctionType.Identity,
                    bias=cb_sb[:, jm : jm + 1],
                )

            # ---- main loop over u, grouped for batched output stores ---------
            u = 0
            for g, gs in enumerate(GROUPS):
                ob_t = obig_pool.tile(
                    [P, TB, CMAX, V], F32, tag="obig", name=f"obig_{g}"
                )
                for ui in range(gs):
                    act_t = act_pool.tile(
                        [P, JC, T], BF16, tag="act", name=f"act_{u}"
                    )
                    for jc in range(JC):
                        nc.scalar.activation(
                            act_t[:, jc, :],
                            ps_enc[jc][:],
                            mybir.ActivationFunctionType.Tanh,
                            bias=dec_sb[:, jc, u : u + 1],
                        )
                    for tb in range(TB):
                        ps = psum_pool.tile(
                            [P, T], F32, tag="ps", name=f"ps_{u}_{tb}"
                        )
                        for jc in range(JC):
                            nc.tensor.matmul(
                                ps[:, :V],
                                lhsT=act_t[:, jc, ts(tb, P)],
                                rhs=Wo_sb[:, jc, :],
                                start=(jc == 0),
                                stop=(jc == JC - 1),
                            )
                        nc.vector.tensor_add(
                            ob_t[:, tb, ui, :], ps[:, :V], ob_sb[:]
                        )
                    u += 1
                # store the group: per t row, gs*2000 contiguous bytes in HBM;
                # alternate issue between the two HWDGE rings.
                for tb in range(TB):
                    eng = nc.sync if tb % 2 == 0 else nc.scalar
                    eng.dma_start(
                        out[ds(tb * P, P), ds(u - gs, gs), :],
                        ob_t[:, tb, 0:gs, :],
                    )

    nc.compile()
    return nc


_CACHED_NC = None


def _get_nc():
    global _CACHED_NC
    if _CACHED_NC is None:
        _CACHED_NC = build_nc()
    return _CACHED_NC


def make_in_maps(
    encoder_out, decoder_out, enc_W, enc_b, dec_W, dec_b, out_W, out_b
) -> list[dict]:
    bf = ml_dtypes.bfloat16
    f32 = np.float32

    def t_bf16(a):  # transpose last two dims, contiguous, bf16
        return np.ascontiguousarray(np.asarray(a, dtype=f32).T).astype(bf)

    WeT = t_bf16(enc_W)  # [E, J]
    WdT = t_bf16(dec_W)  # [D, J]
    WoT = t_bf16(out_W)  # [J, V]
    cb = (np.asarray(enc_b, f32) + np.asarray(dec_b, f32)).astype(f32)  # [J]
    ob = np.ascontiguousarray(
        np.broadcast_to(np.asarray(out_b, f32), (P, V))
    )  # [P, V]

    encoder_out = np.asarray(encoder_out, f32)
    decoder_out = np.asarray(decoder_out, f32)
    in_maps = []
    for i in range(NUM_CORES):
        in_maps.append(
            {
                "xT": t_bf16(encoder_out[i]),  # [E, T]
                "dT": t_bf16(decoder_out[i]),  # [D, U]
                "WeT": WeT,
                "WdT": WdT,
                "WoT": WoT,
                "cb": cb,
                "ob": ob,
            }
        )
    return in_maps


def run(inputs: dict, trace: bool = False):
    """Returns (full_output, BassKernelResults)."""
    nc = _get_nc()
    in_maps = make_in_maps(**inputs)
    res = run_bass_kernel_spmd(
        nc, in_maps, core_ids=list(range(NUM_CORES)), trace=trace
    )
    out = np.stack(
        [res.results[i]["out"] for i in range(NUM_CORES)], axis=0
    )  # (N, T, U, V)
    return np.asarray(out, np.float32), res


def kernel(**inputs) -> np.ndarray:
    out, _ = run(inputs, trace=False)
    return out


# revision 6
# speedup vs baseline: 1.0090x; 1.0090x over previous
"""RNN-T Joiner kernel for Trainium2, data-parallel over batch N across 8 NeuronCores.

Per core (one batch element):
  enc_T[J,T] = enc_W @ x_enc.T          (projection, bf16 matmul, fp32 accum, stays in PSUM)
  dec_T[J,U] = dec_W @ x_dec.T + (enc_b+dec_b)
  act[J,(u,t)] = tanh(enc_T[:,t] + dec_T[:,u])   (ScalarE reads enc from PSUM, dec as bias)
  out[t,u,:] = act.T @ out_W.T + out_b           (PE, act stationary, fp32 PSUM)

Scheduling notes (from trace analysis):
- each dma_start costs ~620ns of serialized issue time on its sequencer; TRN2 has
  two HWDGE rings (sync + scalar), so input/output DMA issues are split across both.
- output rows for one t are contiguous across u in HBM, so staging C consecutive
  u-iterations in SBUF lets one descriptor cover C*2000 bytes; group sizes taper
  ([2,2,4,8*6,4,2,1,1]) so stores start early and the post-matmul drain is ~1 u.
"""

import sys

import numpy as np

try:
    import concourse.bass as bass
except ImportError:
    sys.path.insert(0, "/opt/trn_rl_repo")
    import concourse.bass as bass

import ml_dtypes

import concourse.mybir as mybir
import concourse.tile as tile
from concourse import bacc
from concourse.bass import ds, ts
from concourse.bass_utils import run_bass_kernel_spmd

N, T, U = 8, 512, 64
E = D = J = 512
V = 500
P = 128
JC = J // P  # 4 chunks of J on partitions
TB = T // P  # 4 blocks of T rows per output tile
F32 = mybir.dt.float32
BF16 = mybir.dt.bfloat16

NUM_CORES = 8

# u-iterations per output staging group: start small so stores begin early,
# 4 in steady state (8KB contiguous per descriptor), taper so the final
# drain after the last matmul is a single u (1MB across 4 DMAs).
GROUPS = [2, 2] + [4] * 14 + [2, 1, 1]
assert sum(GROUPS) == U
CMAX = max(GROUPS)


def build_nc() -> bass.Bass:
    nc = bacc.Bacc(
        "TRN2", target_bir_lowering=False, debug=False, num_devices=NUM_CORES
    )
    xT = nc.declare_dram_parameter("xT", [E, T], BF16, isOutput=False)
    dT = nc.declare_dram_parameter("dT", [D, U], BF16, isOutput=False)
    WeT = nc.declare_dram_parameter("WeT", [E, J], BF16, isOutput=False)
    WdT = nc.declare_dram_parameter("WdT", [D, J], BF16, isOutput=False)
    WoT = nc.declare_dram_parameter("WoT", [J, V], BF16, isOutput=False)
    cb = nc.declare_dram_parameter("cb", [J], F32, isOutput=False)
    ob = nc.declare_dram_parameter("ob", [P, V], F32, isOutput=False)
    out = nc.declare_dram_parameter("out", [T, U, V], F32, isOutput=True)

    with tile.TileContext(nc) as tc:
        with (
            tc.tile_pool(name="const", bufs=1) as const_pool,
            tc.tile_pool(name="acts", bufs=3) as act_pool,
            tc.tile_pool(name="obig", bufs=3) as obig_pool,
            tc.tile_pool(name="psenc", bufs=4, space="PSUM") as enc_psum_pool,
            tc.tile_pool(name="psum", bufs=4, space="PSUM") as psum_pool,
        ):
            # ---- persistent SBUF tensors -------------------------------------
            # chunked layouts: [P, chunk, free]
            xT_sb = const_pool.tile([P, E // P, T], BF16, tag="xT")
            dT_sb = const_pool.tile([P, D // P, U], BF16, tag="dT")
            WeT_sb = const_pool.tile([P, E // P, J], BF16, tag="WeT")
            WdT_sb = const_pool.tile([P, D // P, J], BF16, tag="WdT")
            Wo_sb = const_pool.tile([P, JC, V], BF16, tag="WoT")
            cb_sb = const_pool.tile([P, JC], F32, tag="cb")
            ob_sb = const_pool.tile([P, V], F32, tag="ob")
            dec_sb = const_pool.tile([P, JC, U], F32, tag="decT")
            warm_sb = const_pool.tile([P, 64], BF16, tag="warm")

            nc.vector.memset(warm_sb[:], 0.0)

            # ---- input DMAs: coarse transfers, issue split across the two
            # HWDGE rings (sync + scalar); each issue serializes ~620ns on its
            # sequencer, so order = consumption order.
            xT_r = xT.rearrange("(c p) t -> p c t", p=P)
            WeT_r = WeT.rearrange("(c p) j -> p c j", p=P)
            Wo_r = WoT.rearrange("(c p) v -> p c v", p=P)
            # sync ring: enc-projection operands (first chunk alone so the
            # first matmuls can start as early as possible)
            nc.sync.dma_start(xT_sb[:, 0, :], xT_r[:, 0, :])
            nc.sync.dma_start(WeT_sb[:, 0, :], WeT_r[:, 0, :])
            nc.sync.dma_start(xT_sb[:, 1:4, :], xT_r[:, 1:4, :])
            nc.sync.dma_start(WeT_sb[:, 1:4, :], WeT_r[:, 1:4, :])
            # scalar ring: dec-projection operands + output weights/biases
            nc.scalar.dma_start(WdT_sb[:], WdT.rearrange("(c p) j -> p c j", p=P))
            nc.scalar.dma_start(dT_sb[:], dT.rearrange("(c p) u -> p c u", p=P))
            nc.scalar.dma_start(Wo_sb[:, 0:2, :], Wo_r[:, 0:2, :])
            nc.scalar.dma_start(cb_sb[:], cb.rearrange("(c p) -> p c", p=P))
            nc.scalar.dma_start(Wo_sb[:, 2:4, :], Wo_r[:, 2:4, :])
            nc.scalar.dma_start(ob_sb[:], ob[:])

            # ---- projections -------------------------------------------------
            # enc stays in PSUM for the whole kernel (4 banks pinned); tanh
            # reads it directly with dec as the per-partition bias.
            ps_enc = [
                enc_psum_pool.tile([P, T], F32, tag="pse", name=f"ps_enc_{jm}")
                for jm in range(JC)
            ]
            ps_dec = [
                psum_pool.tile([P, T], F32, tag="ps", name=f"ps_dec_{jm}")
                for jm in range(JC)
            ]

            # PE warm-up: dummy matmuls while the input DMAs land. Needs >3.4us
            # of sustained PE busy for the HAM clock-gate to lift (1.2 -> 2.4
            # GHz) before the projections, and keeps the PE busy until the
            # first projection operands arrive (~4.5us in: DMA issue+latency).
            # enc's start=True resets the bank afterwards.
            for w in range(80):
                nc.tensor.matmul(
                    ps_enc[0][:64, :64],
                    lhsT=warm_sb[:, :64],
                    rhs=warm_sb[:],
                    start=True,
                    stop=True,
                    skip_group_check=True,
                )

            # ek outer so the first matmuls only need chunk 0 of the DMAs;
            # enc and dec interleaved per chunk so each DMA round feeds both.
            for ek in range(E // P):
                for jm in range(JC):
                    nc.tensor.matmul(
                        ps_enc[jm][:],
                        lhsT=WeT_sb[:, ek, ts(jm, P)],
                        rhs=xT_sb[:, ek, :],
                        start=(ek == 0),
                        stop=(ek == E // P - 1),
                        skip_group_check=True,
                    )
                for jm in range(JC):
                    nc.tensor.matmul(
                        ps_dec[jm][:, :U],
                        lhsT=WdT_sb[:, ek, ts(jm, P)],
                        rhs=dT_sb[:, ek, :],
                        start=(ek == 0),
                        stop=(ek == E // P - 1),
                        skip_group_check=True,
                    )
            for jm in range(JC):
                nc.scalar.activation(
                    dec_sb[:, jm, :],
                    ps_dec[jm][:, :U],
                    mybir.ActivationFunctionType.Identity,
                    bias=cb_sb[:, jm : jm + 1],
                )

            # ---- main loop over u, grouped for batched output stores ---------
            u = 0
            for g, gs in enumerate(GROUPS):
                ob_t = obig_pool.tile(
                    [P, TB, CMAX, V], F32, tag="obig", name=f"obig_{g}"
                )
                for ui in range(gs):
                    act_t = act_pool.tile(
                        [P, JC, T], BF16, tag="act", name=f"act_{u}"
                    )
                    for jc in range(JC):
                        nc.scalar.activation(
                            act_t[:, jc, :],
                            ps_enc[jc][:],
                            mybir.ActivationFunctionType.Tanh,
                            bias=dec_sb[:, jc, u : u + 1],
                        )
                    for tb in range(TB):
                        ps = psum_pool.tile(
                            [P, T], F32, tag="ps", name=f"ps_{u}_{tb}"
                        )
                        for jc in range(JC):
                            nc.tensor.matmul(
                                ps[:, :V],
                                lhsT=act_t[:, jc, ts(tb, P)],
                                rhs=Wo_sb[:, jc, :],
                                start=(jc == 0),
                                stop=(jc == JC - 1),
                            )
                        nc.vector.tensor_add(
                            ob_t[:, tb, ui, :], ps[:, :V], ob_sb[:]
                        )
                    u += 1
                # store the group: per t row, gs*2000 contiguous bytes in HBM.
                # Mid-stream all stores issue on sync (it is otherwise idle, and
                # a blocking store on scalar would stall the tanh pipeline); the
                # last two groups split across both rings since scalar is done.
                for tb in range(TB):
                    if g >= len(GROUPS) - 2 and tb % 2 == 1:
                        eng = nc.scalar
                    else:
                        eng = nc.sync
                    eng.dma_start(
                        out[ds(tb * P, P), ds(u - gs, gs), :],
                        ob_t[:, tb, 0:gs, :],
                    )

    nc.compile()
    return nc


_CACHED_NC = None


def _get_nc():
    global _CACHED_NC
    if _CACHED_NC is None:
        _CACHED_NC = build_nc()
    return _CACHED_NC


def make_in_maps(
    encoder_out, decoder_out, enc_W, enc_b, dec_W, dec_b, out_W, out_b
) -> list[dict]:
    bf = ml_dtypes.bfloat16
    f32 = np.float32

    def t_bf16(a):  # transpose last two dims, contiguous, bf16
        return np.ascontiguousarray(np.asarray(a, dtype=f32).T).astype(bf)

    WeT = t_bf16(enc_W)  # [E, J]
    WdT = t_bf16(dec_W)  # [D, J]
    WoT = t_bf16(out_W)  # [J, V]
    cb = (np.asarray(enc_b, f32) + np.asarray(dec_b, f32)).astype(f32)  # [J]
    ob = np.ascontiguousarray(
        np.broadcast_to(np.asarray(out_b, f32), (P, V))
    )  # [P, V]

    encoder_out = np.asarray(encoder_out, f32)
    decoder_out = np.asarray(decoder_out, f32)
    in_maps = []
    for i in range(NUM_CORES):
        in_maps.append(
            {
                "xT": t_bf16(encoder_out[i]),  # [E, T]
                "dT": t_bf16(decoder_out[i]),  # [D, U]
                "WeT": WeT,
                "WdT": WdT,
                "WoT": WoT,
                "cb": cb,
                "ob": ob,
            }
        )
    return in_maps


def run(inputs: dict, trace: bool = False):
    """Returns (full_output, BassKernelResults)."""
    nc = _get_nc()
    in_maps = make_in_maps(**inputs)
    res = run_bass_kernel_spmd(
        nc, in_maps, core_ids=list(range(NUM_CORES)), trace=trace
    )
    out = np.stack(
        [res.results[i]["out"] for i in range(NUM_CORES)], axis=0
    )  # (N, T, U, V)
    return np.asarray(out, np.float32), res


def kernel(**inputs) -> np.ndarray:
    out, _ = run(inputs, trace=False)
    return out


# revision 11
# speedup vs baseline: 1.0154x; 1.0064x over previous
"""RNN-T Joiner kernel for Trainium2, data-parallel over batch N across 8 NeuronCores.

Per core (one batch element):
  enc_T[J,T] = enc_W @ x_enc.T          (projection, bf16 matmul, fp32 accum, stays in PSUM)
  dec_T[J,U] = dec_W @ x_dec.T + (enc_b+dec_b)
  act[J,(u,t)] = tanh(enc_T[:,t] + dec_T[:,u])   (ScalarE reads enc from PSUM, dec as bias)
  out[t,u,:] = act.T @ out_W.T + out_b           (PE, act stationary, fp32 PSUM)

Scheduling notes (from trace analysis):
- each dma_start costs ~620ns of serialized issue time on its sequencer; TRN2 has
  two HWDGE rings (sync + scalar), so input/output DMA issues are split across both.
- output rows for one t are contiguous across u in HBM, so staging C consecutive
  u-iterations in SBUF lets one descriptor cover C*2000 bytes; group sizes taper
  ([2,2,4,8*6,4,2,1,1]) so stores start early and the post-matmul drain is ~1 u.
"""

import sys

import numpy as np

try:
    import concourse.bass as bass
except ImportError:
    sys.path.insert(0, "/opt/trn_rl_repo")
    import concourse.bass as bass

import ml_dtypes

import concourse.mybir as mybir
import concourse.tile as tile
from concourse import bacc
from concourse.bass import ds, ts
from concourse.bass_utils import run_bass_kernel_spmd

N, T, U = 8, 512, 64
E = D = J = 512
V = 500
P = 128
JC = J // P  # 4 chunks of J on partitions
TB = T // P  # 4 blocks of T rows per output tile
F32 = mybir.dt.float32
F16 = mybir.dt.float16
BF16 = mybir.dt.bfloat16

NUM_CORES = 8

# u-iterations per output staging group: start small so stores begin early,
# 4 in steady state (8KB contiguous per descriptor), taper so the final
# drain after the last matmul is a single u (1MB across 4 DMAs).
GROUPS = [2, 2] + [4] * 14 + [2, 1, 1]
assert sum(GROUPS) == U
CMAX = max(GROUPS)


def build_nc() -> bass.Bass:
    nc = bacc.Bacc(
        "TRN2", target_bir_lowering=False, debug=False, num_devices=NUM_CORES
    )
    xT = nc.declare_dram_parameter("xT", [E, T], BF16, isOutput=False)
    dT = nc.declare_dram_parameter("dT", [D, U], BF16, isOutput=False)
    WeT = nc.declare_dram_parameter("WeT", [E, J], BF16, isOutput=False)
    WdT = nc.declare_dram_parameter("WdT", [D, J], BF16, isOutput=False)
    WoT = nc.declare_dram_parameter("WoT", [J, V], BF16, isOutput=False)
    cb = nc.declare_dram_parameter("cb", [J], F32, isOutput=False)
    ob = nc.declare_dram_parameter("ob", [P, V], F32, isOutput=False)
    # fp16 output: halves HBM store traffic (the binding constraint at ~290
    # GB/s/core effective write bandwidth); host upcasts to f32. Logits absmax
    # ~4.6 so fp16 quantization adds only ~5e-4 relative error.
    out = nc.declare_dram_parameter("out", [T, U, V], F16, isOutput=True)

    with tile.TileContext(nc) as tc:
        with (
            tc.tile_pool(name="const", bufs=1) as const_pool,
            tc.tile_pool(name="acts", bufs=3) as act_pool,
            tc.tile_pool(name="obig", bufs=3) as obig_pool,
            tc.tile_pool(name="psenc", bufs=4, space="PSUM") as enc_psum_pool,
            tc.tile_pool(name="psum", bufs=4, space="PSUM") as psum_pool,
        ):
            # ---- persistent SBUF tensors -------------------------------------
            # chunked layouts: [P, chunk, free]
            xT_sb = const_pool.tile([P, E // P, T], BF16, tag="xT")
            dT_sb = const_pool.tile([P, D // P, U], BF16, tag="dT")
            WeT_sb = const_pool.tile([P, E // P, J], BF16, tag="WeT")
            WdT_sb = const_pool.tile([P, D // P, J], BF16, tag="WdT")
            Wo_sb = const_pool.tile([P, JC, V], BF16, tag="WoT")
            cb_sb = const_pool.tile([P, JC], F32, tag="cb")
            ob_sb = const_pool.tile([P, V], F32, tag="ob")
            dec_sb = const_pool.tile([P, JC, U], F32, tag="decT")
            warm_sb = const_pool.tile([P, 64], BF16, tag="warm")

            nc.vector.memset(warm_sb[:], 0.0)

            # ---- input DMAs: coarse transfers, issue split across the two
            # HWDGE rings (sync + scalar); each issue serializes ~620ns on its
            # sequencer, so order = consumption order.
            xT_r = xT.rearrange("(c p) t -> p c t", p=P)
            WeT_r = WeT.rearrange("(c p) j -> p c j", p=P)
            Wo_r = WoT.rearrange("(c p) v -> p c v", p=P)
            # xT on sync, WeT on scalar so each enc chunk's two operands land
            # together via parallel HWDGE rings; dec operands on gpsimd
            # (SWDGE, third parallel path); Wo split across both HWDGE rings.
            nc.sync.dma_start(xT_sb[:, 0, :], xT_r[:, 0, :])
            nc.scalar.dma_start(WeT_sb[:, 0, :], WeT_r[:, 0, :])
            nc.sync.dma_start(xT_sb[:, 1:4, :], xT_r[:, 1:4, :])
            nc.scalar.dma_start(WeT_sb[:, 1:4, :], WeT_r[:, 1:4, :])
            nc.gpsimd.dma_start(WdT_sb[:], WdT.rearrange("(c p) j -> p c j", p=P))
            nc.gpsimd.dma_start(dT_sb[:], dT.rearrange("(c p) u -> p c u", p=P))
            nc.gpsimd.dma_start(cb_sb[:], cb.rearrange("(c p) -> p c", p=P))
            nc.sync.dma_start(Wo_sb[:, 0:2, :], Wo_r[:, 0:2, :])
            nc.scalar.dma_start(Wo_sb[:, 2:4, :], Wo_r[:, 2:4, :])
            nc.sync.dma_start(ob_sb[:], ob[:])

            # ---- projections -------------------------------------------------
            # enc stays in PSUM for the whole kernel (4 banks pinned); tanh
            # reads it directly with dec as the per-partition bias.
            ps_enc = [
                enc_psum_pool.tile([P, T], F32, tag="pse", name=f"ps_enc_{jm}")
                for jm in range(JC)
            ]
            ps_dec = [
                psum_pool.tile([P, T], F32, tag="ps", name=f"ps_dec_{jm}")
                for jm in range(JC)
            ]

            # PE warm-up: dummy matmuls while the input DMAs land. Needs >3.4us
            # of sustained PE busy for the HAM clock-gate to lift (1.2 -> 2.4
            # GHz) before the projections, and keeps the PE busy until the
            # first projection operands arrive (~4.5us in: DMA issue+latency).
            # enc's start=True resets the bank afterwards.
            for w in range(56):
                nc.tensor.matmul(
                    ps_enc[0][:64, :64],
                    lhsT=warm_sb[:, :64],
                    rhs=warm_sb[:],
                    start=True,
                    stop=True,
                    skip_group_check=True,
                )

            # ek outer so the first matmuls only need chunk 0 of the DMAs;
            # enc and dec interleaved per chunk so each DMA round feeds both.
            for ek in range(E // P):
                for jm in range(JC):
                    nc.tensor.matmul(
                        ps_enc[jm][:],
                        lhsT=WeT_sb[:, ek, ts(jm, P)],
                        rhs=xT_sb[:, ek, :],
                        start=(ek == 0),
                        stop=(ek == E // P - 1),
                        skip_group_check=True,
                    )
                for jm in range(JC):
                    nc.tensor.matmul(
                        ps_dec[jm][:, :U],
                        lhsT=WdT_sb[:, ek, ts(jm, P)],
                        rhs=dT_sb[:, ek, :],
                        start=(ek == 0),
                        stop=(ek == E // P - 1),
                        skip_group_check=True,
                    )
            for jm in range(JC):
                nc.scalar.activation(
                    dec_sb[:, jm, :],
                    ps_dec[jm][:, :U],
                    mybir.ActivationFunctionType.Identity,
                    bias=cb_sb[:, jm : jm + 1],
                )

            # ---- main loop over u, grouped for batched output stores ---------
            u = 0
            for g, gs in enumerate(GROUPS):
                ob_t = obig_pool.tile(
                    [P, TB, CMAX, V], F16, tag="obig", name=f"obig_{g}"
                )
                for ui in range(gs):
                    act_t = act_pool.tile(
                        [P, JC, T], BF16, tag="act", name=f"act_{u}"
                    )
                    for jc in range(JC):
                        nc.scalar.activation(
                            act_t[:, jc, :],
                            ps_enc[jc][:],
                            mybir.ActivationFunctionType.Tanh,
                            bias=dec_sb[:, jc, u : u + 1],
                        )
                    for tb in range(TB):
                        ps = psum_pool.tile(
                            [P, T], F32, tag="ps", name=f"ps_{u}_{tb}"
                        )
                        for jc in range(JC):
                            nc.tensor.matmul(
                                ps[:, :V],
                                lhsT=act_t[:, jc, ts(tb, P)],
                                rhs=Wo_sb[:, jc, :],
                                start=(jc == 0),
                                stop=(jc == JC - 1),
                            )
                        nc.vector.tensor_add(
                            ob_t[:, tb, ui, :], ps[:, :V], ob_sb[:]
                        )
                    u += 1
                # store the group: per t row, gs*2000 contiguous bytes in HBM.
                # Mid-stream all stores issue on sync (it is otherwise idle, and
                # a blocking store on scalar would stall the tanh pipeline); the
                # last two groups split across both rings since scalar is done.
                for tb in range(TB):
                    if g >= len(GROUPS) - 2 and tb % 2 == 1:
                        eng = nc.scalar
                    else:
                        eng = nc.sync
                    eng.dma_start(
                        out[ds(tb * P, P), ds(u - gs, gs), :],
                        ob_t[:, tb, 0:gs, :],
                    )

    nc.compile()
    return nc


_CACHED_NC = None


def _get_nc():
    global _CACHED_NC
    if _CACHED_NC is None:
        _CACHED_NC = build_nc()
    return _CACHED_NC


def make_in_maps(
    encoder_out, decoder_out, enc_W, enc_b, dec_W, dec_b, out_W, out_b
) -> list[dict]:
    bf = ml_dtypes.bfloat16
    f32 = np.float32

    def t_bf16(a):  # transpose last two dims, contiguous, bf16
        return np.ascontiguousarray(np.asarray(a, dtype=f32).T).astype(bf)

    WeT = t_bf16(enc_W)  # [E, J]
    WdT = t_bf16(dec_W)  # [D, J]
    WoT = t_bf16(out_W)  # [J, V]
    cb = (np.asarray(enc_b, f32) + np.asarray(dec_b, f32)).astype(f32)  # [J]
    ob = np.ascontiguousarray(
        np.broadcast_to(np.asarray(out_b, f32), (P, V))
    )  # [P, V]

    encoder_out = np.asarray(encoder_out, f32)
    decoder_out = np.asarray(decoder_out, f32)
    in_maps = []
    for i in range(NUM_CORES):
        in_maps.append(
            {
                "xT": t_bf16(encoder_out[i]),  # [E, T]
                "dT": t_bf16(decoder_out[i]),  # [D, U]
                "WeT": WeT,
                "WdT": WdT,
                "WoT": WoT,
                "cb": cb,
                "ob": ob,
            }
        )
    return in_maps


def run(inputs: dict, trace: bool = False):
    """Returns (full_output, BassKernelResults)."""
    nc = _get_nc()
    in_maps = make_in_maps(**inputs)
    res = run_bass_kernel_spmd(
        nc, in_maps, core_ids=list(range(NUM_CORES)), trace=trace
    )
    out = np.stack(
        [res.results[i]["out"] for i in range(NUM_CORES)], axis=0
    )  # (N, T, U, V)
    return np.asarray(out, np.float32), res


def kernel(**inputs) -> np.ndarray:
    out, _ = run(inputs, trace=False)
    return out


# revision 13
# speedup vs baseline: 1.0363x; 1.0206x over previous
"""RNN-T Joiner kernel for Trainium2, data-parallel over batch N across 8 NeuronCores.

Per core (one batch element):
  enc_T[J,T] = enc_W @ x_enc.T          (projection, bf16 matmul, fp32 accum, stays in PSUM)
  dec_T[J,U] = dec_W @ x_dec.T + (enc_b+dec_b)
  act[J,(u,t)] = tanh(enc_T[:,t] + dec_T[:,u])   (ScalarE reads enc from PSUM, dec as bias)
  out[t,u,:] = act.T @ out_W.T + out_b           (PE, act stationary, fp32 PSUM)

Scheduling notes (from trace analysis):
- each dma_start costs ~620ns of serialized issue time on its sequencer; TRN2 has
  two HWDGE rings (sync + scalar), so input/output DMA issues are split across both.
- output rows for one t are contiguous across u in HBM, so staging C consecutive
  u-iterations in SBUF lets one descriptor cover C*2000 bytes; group sizes taper
  ([2,2,4,8*6,4,2,1,1]) so stores start early and the post-matmul drain is ~1 u.
"""

import sys

import numpy as np

try:
    import concourse.bass as bass
except ImportError:
    sys.path.insert(0, "/opt/trn_rl_repo")
    import concourse.bass as bass

import ml_dtypes

import concourse.mybir as mybir
import concourse.tile as tile
from concourse import bacc
from concourse.bass import ds, ts
from concourse.bass_utils import run_bass_kernel_spmd

N, T, U = 8, 512, 64
E = D = J = 512
V = 500
P = 128
JC = J // P  # 4 chunks of J on partitions
TB = T // P  # 4 blocks of T rows per output tile
F32 = mybir.dt.float32
F16 = mybir.dt.float16
BF16 = mybir.dt.bfloat16

NUM_CORES = 8

# u-iterations per output staging group: start small so stores begin early,
# 4 in steady state (8KB contiguous per descriptor), taper so the final
# drain after the last matmul is a single u (1MB across 4 DMAs).
GROUPS = [2, 2] + [4] * 14 + [2, 1, 1]
assert sum(GROUPS) == U
CMAX = max(GROUPS)


def build_nc() -> bass.Bass:
    nc = bacc.Bacc(
        "TRN2", target_bir_lowering=False, debug=False, num_devices=NUM_CORES
    )
    xT = nc.declare_dram_parameter("xT", [E, T], BF16, isOutput=False)
    dT = nc.declare_dram_parameter("dT", [D, U], BF16, isOutput=False)
    WeT = nc.declare_dram_parameter("WeT", [E, J], BF16, isOutput=False)
    WdT = nc.declare_dram_parameter("WdT", [D, J], BF16, isOutput=False)
    WoT = nc.declare_dram_parameter("WoT", [J, V], BF16, isOutput=False)
    cb = nc.declare_dram_parameter("cb", [J], F32, isOutput=False)
    ob = nc.declare_dram_parameter("ob", [P, V], F32, isOutput=False)
    # fp16 output: halves HBM store traffic (the binding constraint at ~290
    # GB/s/core effective write bandwidth); host upcasts to f32. Logits absmax
    # ~4.6 so fp16 quantization adds only ~5e-4 relative error.
    out = nc.declare_dram_parameter("out", [T, U, V], F16, isOutput=True)

    with tile.TileContext(nc) as tc:
        with (
            tc.tile_pool(name="const", bufs=1) as const_pool,
            tc.tile_pool(name="acts", bufs=3) as act_pool,
            tc.tile_pool(name="obig", bufs=3) as obig_pool,
            tc.tile_pool(name="psenc", bufs=4, space="PSUM") as enc_psum_pool,
            tc.tile_pool(name="psum", bufs=4, space="PSUM") as psum_pool,
        ):
            # ---- persistent SBUF tensors -------------------------------------
            # chunked layouts: [P, chunk, free]
            xT_sb = const_pool.tile([P, E // P, T], BF16, tag="xT")
            dT_sb = const_pool.tile([P, D // P, U], BF16, tag="dT")
            WeT_sb = const_pool.tile([P, E // P, J], BF16, tag="WeT")
            WdT_sb = const_pool.tile([P, D // P, J], BF16, tag="WdT")
            Wo_sb = const_pool.tile([P, JC, V], BF16, tag="WoT")
            cb_sb = const_pool.tile([P, JC], F32, tag="cb")
            ob_sb = const_pool.tile([P, V], F32, tag="ob")
            dec_sb = const_pool.tile([P, JC, U], F32, tag="decT")
            warm_sb = const_pool.tile([P, 64], BF16, tag="warm")

            nc.vector.memset(warm_sb[:], 0.0)

            # ---- input DMAs: coarse transfers, issue split across the two
            # HWDGE rings (sync + scalar); each issue serializes ~620ns on its
            # sequencer, so order = consumption order.
            xT_r = xT.rearrange("(c p) t -> p c t", p=P)
            WeT_r = WeT.rearrange("(c p) j -> p c j", p=P)
            Wo_r = WoT.rearrange("(c p) v -> p c v", p=P)
            # xT on sync, WeT on scalar so each enc chunk's two operands land
            # together via parallel HWDGE rings (gpsimd SWDGE adds ~10us
            # latency -- never put time-critical inputs there). Issue order =
            # consumption order: enc chunks, dec operands, Wo, biases.
            nc.sync.dma_start(xT_sb[:, 0, :], xT_r[:, 0, :])
            nc.scalar.dma_start(WeT_sb[:, 0, :], WeT_r[:, 0, :])
            nc.sync.dma_start(xT_sb[:, 1:4, :], xT_r[:, 1:4, :])
            nc.scalar.dma_start(WdT_sb[:], WdT.rearrange("(c p) j -> p c j", p=P))
            nc.scalar.dma_start(WeT_sb[:, 1:4, :], WeT_r[:, 1:4, :])
            nc.sync.dma_start(Wo_sb[:, 0:2, :], Wo_r[:, 0:2, :])
            nc.scalar.dma_start(dT_sb[:], dT.rearrange("(c p) u -> p c u", p=P))
            nc.scalar.dma_start(cb_sb[:], cb.rearrange("(c p) -> p c", p=P))
            nc.scalar.dma_start(Wo_sb[:, 2:4, :], Wo_r[:, 2:4, :])
            nc.sync.dma_start(ob_sb[:], ob[:])

            # ---- projections -------------------------------------------------
            # enc stays in PSUM for the whole kernel (4 banks pinned); tanh
            # reads it directly with dec as the per-partition bias.
            ps_enc = [
                enc_psum_pool.tile([P, T], F32, tag="pse", name=f"ps_enc_{jm}")
                for jm in range(JC)
            ]
            ps_dec = [
                psum_pool.tile([P, T], F32, tag="ps", name=f"ps_dec_{jm}")
                for jm in range(JC)
            ]

            # PE warm-up: dummy matmuls while the input DMAs land. Needs >3.4us
            # of sustained PE busy for the HAM clock-gate to lift (1.2 -> 2.4
            # GHz) before the projections, and keeps the PE busy until the
            # first projection operands arrive (~4.5us in: DMA issue+latency).
            # enc's start=True resets the bank afterwards.
            for w in range(56):
                nc.tensor.matmul(
                    ps_enc[0][:64, :64],
                    lhsT=warm_sb[:, :64],
                    rhs=warm_sb[:],
                    start=True,
                    stop=True,
                    skip_group_check=True,
                )

            # ek outer so the first matmuls only need chunk 0 of the DMAs;
            # all enc first (its inputs arrive first), then all dec.
            for ek in range(E // P):
                for jm in range(JC):
                    nc.tensor.matmul(
                        ps_enc[jm][:],
                        lhsT=WeT_sb[:, ek, ts(jm, P)],
                        rhs=xT_sb[:, ek, :],
                        start=(ek == 0),
                        stop=(ek == E // P - 1),
                        skip_group_check=True,
                    )
            for ek in range(E // P):
                for jm in range(JC):
                    nc.tensor.matmul(
                        ps_dec[jm][:, :U],
                        lhsT=WdT_sb[:, ek, ts(jm, P)],
                        rhs=dT_sb[:, ek, :],
                        start=(ek == 0),
                        stop=(ek == E // P - 1),
                        skip_group_check=True,
                    )
            for jm in range(JC):
                nc.scalar.activation(
                    dec_sb[:, jm, :],
                    ps_dec[jm][:, :U],
                    mybir.ActivationFunctionType.Identity,
                    bias=cb_sb[:, jm : jm + 1],
                )

            # ---- main loop over u, grouped for batched output stores ---------
            u = 0
            for g, gs in enumerate(GROUPS):
                ob_t = obig_pool.tile(
                    [P, TB, CMAX, V], F16, tag="obig", name=f"obig_{g}"
                )
                for ui in range(gs):
                    act_t = act_pool.tile(
                        [P, JC, T], BF16, tag="act", name=f"act_{u}"
                    )
                    for jc in range(JC):
                        nc.scalar.activation(
                            act_t[:, jc, :],
                            ps_enc[jc][:],
                            mybir.ActivationFunctionType.Tanh,
                            bias=dec_sb[:, jc, u : u + 1],
                        )
                    for tb in range(TB):
                        ps = psum_pool.tile(
                            [P, T], F32, tag="ps", name=f"ps_{u}_{tb}"
                        )
                        for jc in range(JC):
                            nc.tensor.matmul(
                                ps[:, :V],
                                lhsT=act_t[:, jc, ts(tb, P)],
                                rhs=Wo_sb[:, jc, :],
                                start=(jc == 0),
                                stop=(jc == JC - 1),
                            )
                        nc.vector.tensor_add(
                            ob_t[:, tb, ui, :], ps[:, :V], ob_sb[:]
                        )
                    u += 1
                # store the group: per t row, gs*2000 contiguous bytes in HBM.
                # Mid-stream all stores issue on sync (it is otherwise idle, and
                # a blocking store on scalar would stall the tanh pipeline); the
                # last two groups split across both rings since scalar is done.
                for tb in range(TB):
                    if g >= len(GROUPS) - 2 and tb % 2 == 1:
                        eng = nc.scalar
                    else:
                        eng = nc.sync
                    eng.dma_start(
                        out[ds(tb * P, P), ds(u - gs, gs), :],
                        ob_t[:, tb, 0:gs, :],
                    )

    nc.compile()
    return nc


_CACHED_NC = None


def _get_nc():
    global _CACHED_NC
    if _CACHED_NC is None:
        _CACHED_NC = build_nc()
    return _CACHED_NC


def make_in_maps(
    encoder_out, decoder_out, enc_W, enc_b, dec_W, dec_b, out_W, out_b
) -> list[dict]:
    bf = ml_dtypes.bfloat16
    f32 = np.float32

    def t_bf16(a):  # transpose last two dims, contiguous, bf16
        return np.ascontiguousarray(np.asarray(a, dtype=f32).T).astype(bf)

    WeT = t_bf16(enc_W)  # [E, J]
    WdT = t_bf16(dec_W)  # [D, J]
    WoT = t_bf16(out_W)  # [J, V]
    cb = (np.asarray(enc_b, f32) + np.asarray(dec_b, f32)).astype(f32)  # [J]
    ob = np.ascontiguousarray(
        np.broadcast_to(np.asarray(out_b, f32), (P, V))
    )  # [P, V]

    encoder_out = np.asarray(encoder_out, f32)
    decoder_out = np.asarray(decoder_out, f32)
    in_maps = []
    for i in range(NUM_CORES):
        in_maps.append(
            {
                "xT": t_bf16(encoder_out[i]),  # [E, T]
                "dT": t_bf16(decoder_out[i]),  # [D, U]
                "WeT": WeT,
                "WdT": WdT,
                "WoT": WoT,
                "cb": cb,
                "ob": ob,
            }
        )
    return in_maps


def run(inputs: dict, trace: bool = False):
    """Returns (full_output, BassKernelResults)."""
    nc = _get_nc()
    in_maps = make_in_maps(**inputs)
    res = run_bass_kernel_spmd(
        nc, in_maps, core_ids=list(range(NUM_CORES)), trace=trace
    )
    out = np.stack(
        [res.results[i]["out"] for i in range(NUM_CORES)], axis=0
    )  # (N, T, U, V)
    return np.asarray(out, np.float32), res


def kernel(**inputs) -> np.ndarray:
    out, _ = run(inputs, trace=False)
    return out


# revision 15
# speedup vs baseline: 1.0539x; 1.0170x over previous
"""RNN-T Joiner kernel for Trainium2, data-parallel over batch N across 8 NeuronCores.

Per core (one batch element):
  enc_T[J,T] = enc_W @ x_enc.T          (projection, bf16 matmul, fp32 accum, stays in PSUM)
  dec_T[J,U] = dec_W @ x_dec.T + (enc_b+dec_b)
  act[J,(u,t)] = tanh(enc_T[:,t] + dec_T[:,u])   (ScalarE reads enc from PSUM, dec as bias)
  out[t,u,:] = act.T @ out_W.T + out_b           (PE, act stationary, fp32 PSUM)

Scheduling notes (from trace analysis):
- each dma_start costs ~620ns of serialized issue time on its sequencer; TRN2 has
  two HWDGE rings (sync + scalar), so input/output DMA issues are split across both.
- output rows for one t are contiguous across u in HBM, so staging C consecutive
  u-iterations in SBUF lets one descriptor cover C*2000 bytes; group sizes taper
  ([2,2,4,8*6,4,2,1,1]) so stores start early and the post-matmul drain is ~1 u.
"""

import sys

import numpy as np

try:
    import concourse.bass as bass
except ImportError:
    sys.path.insert(0, "/opt/trn_rl_repo")
    import concourse.bass as bass

import ml_dtypes

import concourse.mybir as mybir
import concourse.tile as tile
from concourse import bacc
from concourse.bass import ds, ts
from concourse.bass_utils import run_bass_kernel_spmd

N, T, U = 8, 512, 64
E = D = J = 512
V = 500
P = 128
JC = J // P  # 4 chunks of J on partitions
TB = T // P  # 4 blocks of T rows per output tile
F32 = mybir.dt.float32
F16 = mybir.dt.float16
BF16 = mybir.dt.bfloat16

NUM_CORES = 8

# u-iterations per output staging group: start small so stores begin early,
# 4 in steady state (8KB contiguous per descriptor), taper so the final
# drain after the last matmul is a single u (1MB across 4 DMAs).
GROUPS = [2, 2] + [4] * 14 + [2, 1, 1]
assert sum(GROUPS) == U
CMAX = max(GROUPS)


def build_nc() -> bass.Bass:
    nc = bacc.Bacc(
        "TRN2", target_bir_lowering=False, debug=False, num_devices=NUM_CORES
    )
    xT = nc.declare_dram_parameter("xT", [E, T], BF16, isOutput=False)
    dT = nc.declare_dram_parameter("dT", [D, U], BF16, isOutput=False)
    WeT = nc.declare_dram_parameter("WeT", [E, J], BF16, isOutput=False)
    WdT = nc.declare_dram_parameter("WdT", [D, J], BF16, isOutput=False)
    WoT = nc.declare_dram_parameter("WoT", [J, V], BF16, isOutput=False)
    cb = nc.declare_dram_parameter("cb", [J], F32, isOutput=False)
    ob = nc.declare_dram_parameter("ob", [P, V], F32, isOutput=False)
    # fp16 output: halves HBM store traffic (the binding constraint at ~290
    # GB/s/core effective write bandwidth); host upcasts to f32. Logits absmax
    # ~4.6 so fp16 quantization adds only ~5e-4 relative error.
    out = nc.declare_dram_parameter("out", [T, U, V], F16, isOutput=True)

    with tile.TileContext(nc) as tc:
        with (
            tc.tile_pool(name="const", bufs=1) as const_pool,
            tc.tile_pool(name="acts", bufs=3) as act_pool,
            tc.tile_pool(name="obig", bufs=3) as obig_pool,
            tc.tile_pool(name="psenc", bufs=4, space="PSUM") as enc_psum_pool,
            tc.tile_pool(name="psum", bufs=4, space="PSUM") as psum_pool,
        ):
            # ---- persistent SBUF tensors -------------------------------------
            # chunked layouts: [P, chunk, free]
            xT_sb = const_pool.tile([P, E // P, T], BF16, tag="xT")
            dT_sb = const_pool.tile([P, D // P, U], BF16, tag="dT")
            WeT_sb = const_pool.tile([P, E // P, J], BF16, tag="WeT")
            WdT_sb = const_pool.tile([P, D // P, J], BF16, tag="WdT")
            Wo_sb = const_pool.tile([P, JC, V], BF16, tag="WoT")
            cb_sb = const_pool.tile([P, JC], F32, tag="cb")
            ob_sb = const_pool.tile([P, V], F32, tag="ob")
            dec_sb = const_pool.tile([P, JC, U], F32, tag="decT")
            warm_sb = const_pool.tile([P, 64], BF16, tag="warm")

            nc.vector.memset(warm_sb[:], 0.0)

            # ---- input DMAs: coarse transfers, issue split across the two
            # HWDGE rings (sync + scalar); each issue serializes ~620ns on its
            # sequencer, so order = consumption order.
            xT_r = xT.rearrange("(c p) t -> p c t", p=P)
            WeT_r = WeT.rearrange("(c p) j -> p c j", p=P)
            Wo_r = WoT.rearrange("(c p) v -> p c v", p=P)
            # xT on sync, WeT on scalar so each enc chunk's two operands land
            # together via parallel HWDGE rings (gpsimd SWDGE adds ~10us
            # latency -- never put time-critical inputs there). Concurrent
            # transfers round-robin the HBM bandwidth at packet granularity,
            # so anything issued early steals bandwidth from the enc chunks:
            # strict priority order, Wo deferred behind the projections.
            nc.sync.dma_start(xT_sb[:, 0, :], xT_r[:, 0, :])
            nc.scalar.dma_start(WeT_sb[:, 0, :], WeT_r[:, 0, :])
            nc.sync.dma_start(xT_sb[:, 1:4, :], xT_r[:, 1:4, :])
            nc.scalar.dma_start(WeT_sb[:, 1:4, :], WeT_r[:, 1:4, :])
            nc.scalar.dma_start(WdT_sb[:], WdT.rearrange("(c p) j -> p c j", p=P))
            nc.sync.dma_start(dT_sb[:], dT.rearrange("(c p) u -> p c u", p=P))
            nc.sync.dma_start(cb_sb[:], cb.rearrange("(c p) -> p c", p=P))
            nc.sync.dma_start(Wo_sb[:, 0:2, :], Wo_r[:, 0:2, :])
            nc.scalar.dma_start(Wo_sb[:, 2:4, :], Wo_r[:, 2:4, :])
            nc.sync.dma_start(ob_sb[:], ob[:])

            # ---- projections -------------------------------------------------
            # enc stays in PSUM for the whole kernel (4 banks pinned); tanh
            # reads it directly with dec as the per-partition bias.
            ps_enc = [
                enc_psum_pool.tile([P, T], F32, tag="pse", name=f"ps_enc_{jm}")
                for jm in range(JC)
            ]
            ps_dec = [
                psum_pool.tile([P, T], F32, tag="ps", name=f"ps_dec_{jm}")
                for jm in range(JC)
            ]

            # PE warm-up: dummy matmuls while the input DMAs land. Needs >3.4us
            # of sustained PE busy for the HAM clock-gate to lift (1.2 -> 2.4
            # GHz) before the projections, and keeps the PE busy until the
            # first projection operands arrive (~4.5us in: DMA issue+latency).
            # enc's start=True resets the bank afterwards.
            for w in range(40):
                nc.tensor.matmul(
                    ps_enc[0][:64, :64],
                    lhsT=warm_sb[:, :64],
                    rhs=warm_sb[:],
                    start=True,
                    stop=True,
                    skip_group_check=True,
                )

            # ek outer so the first matmuls only need chunk 0 of the DMAs;
            # all enc first (its inputs arrive first), then all dec.
            for ek in range(E // P):
                for jm in range(JC):
                    nc.tensor.matmul(
                        ps_enc[jm][:],
                        lhsT=WeT_sb[:, ek, ts(jm, P)],
                        rhs=xT_sb[:, ek, :],
                        start=(ek == 0),
                        stop=(ek == E // P - 1),
                        skip_group_check=True,
                    )
            for ek in range(E // P):
                for jm in range(JC):
                    nc.tensor.matmul(
                        ps_dec[jm][:, :U],
                        lhsT=WdT_sb[:, ek, ts(jm, P)],
                        rhs=dT_sb[:, ek, :],
                        start=(ek == 0),
                        stop=(ek == E // P - 1),
                        skip_group_check=True,
                    )
            for jm in range(JC):
                nc.scalar.activation(
                    dec_sb[:, jm, :],
                    ps_dec[jm][:, :U],
                    mybir.ActivationFunctionType.Identity,
                    bias=cb_sb[:, jm : jm + 1],
                )

            # ---- main loop over u, grouped for batched output stores ---------
            u = 0
            for g, gs in enumerate(GROUPS):
                ob_t = obig_pool.tile(
                    [P, TB, CMAX, V], F16, tag="obig", name=f"obig_{g}"
                )
                for ui in range(gs):
                    act_t = act_pool.tile(
                        [P, JC, T], BF16, tag="act", name=f"act_{u}"
                    )
                    for jc in range(JC):
                        nc.scalar.activation(
                            act_t[:, jc, :],
                            ps_enc[jc][:],
                            mybir.ActivationFunctionType.Tanh,
                            bias=dec_sb[:, jc, u : u + 1],
                        )
                    for tb in range(TB):
                        ps = psum_pool.tile(
                            [P, T], F32, tag="ps", name=f"ps_{u}_{tb}"
                        )
                        for jc in range(JC):
                            nc.tensor.matmul(
                                ps[:, :V],
                                lhsT=act_t[:, jc, ts(tb, P)],
                                rhs=Wo_sb[:, jc, :],
                                start=(jc == 0),
                                stop=(jc == JC - 1),
                            )
                        nc.vector.tensor_add(
                            ob_t[:, tb, ui, :], ps[:, :V], ob_sb[:]
                        )
                    u += 1
                # store the group: per t row, gs*2000 contiguous bytes in HBM.
                # Mid-stream all stores issue on sync (it is otherwise idle, and
                # a blocking store on scalar would stall the tanh pipeline); the
                # last two groups split across both rings since scalar is done.
                for tb in range(TB):
                    if g >= len(GROUPS) - 2 and tb % 2 == 1:
                        eng = nc.scalar
                    else:
                        eng = nc.sync
                    eng.dma_start(
                        out[ds(tb * P, P), ds(u - gs, gs), :],
                        ob_t[:, tb, 0:gs, :],
                    )

    nc.compile()
    return nc


_CACHED_NC = None


def _get_nc():
    global _CACHED_NC
    if _CACHED_NC is None:
        _CACHED_NC = build_nc()
    return _CACHED_NC


def make_in_maps(
    encoder_out, decoder_out, enc_W, enc_b, dec_W, dec_b, out_W, out_b
) -> list[dict]:
    bf = ml_dtypes.bfloat16
    f32 = np.float32

    def t_bf16(a):  # transpose last two dims, contiguous, bf16
        return np.ascontiguousarray(np.asarray(a, dtype=f32).T).astype(bf)

    WeT = t_bf16(enc_W)  # [E, J]
    WdT = t_bf16(dec_W)  # [D, J]
    WoT = t_bf16(out_W)  # [J, V]
    cb = (np.asarray(enc_b, f32) + np.asarray(dec_b, f32)).astype(f32)  # [J]
    ob = np.ascontiguousarray(
        np.broadcast_to(np.asarray(out_b, f32), (P, V))
    )  # [P, V]

    encoder_out = np.asarray(encoder_out, f32)
    decoder_out = np.asarray(decoder_out, f32)
    in_maps = []
    for i in range(NUM_CORES):
        in_maps.append(
            {
                "xT": t_bf16(encoder_out[i]),  # [E, T]
                "dT": t_bf16(decoder_out[i]),  # [D, U]
                "WeT": WeT,
                "WdT": WdT,
                "WoT": WoT,
                "cb": cb,
                "ob": ob,
            }
        )
    return in_maps


def run(inputs: dict, trace: bool = False):
    """Returns (full_output, BassKernelResults)."""
    nc = _get_nc()
    in_maps = make_in_maps(**inputs)
    res = run_bass_kernel_spmd(
        nc, in_maps, core_ids=list(range(NUM_CORES)), trace=trace
    )
    out = np.stack(
        [res.results[i]["out"] for i in range(NUM_CORES)], axis=0
    )  # (N, T, U, V)
    return np.asarray(out, np.float32), res


def kernel(**inputs) -> np.ndarray:
    out, _ = run(inputs, trace=False)
    return out


# revision 17
# speedup vs baseline: 1.0567x; 1.0026x over previous
"""RNN-T Joiner kernel for Trainium2, data-parallel over batch N across 8 NeuronCores.

Per core (one batch element):
  enc_T[J,T] = enc_W @ x_enc.T          (projection, bf16 matmul, fp32 accum, stays in PSUM)
  dec_T[J,U] = dec_W @ x_dec.T + (enc_b+dec_b)
  act[J,(u,t)] = tanh(enc_T[:,t] + dec_T[:,u])   (ScalarE reads enc from PSUM, dec as bias)
  out[t,u,:] = act.T @ out_W.T + out_b           (PE, act stationary, fp32 PSUM)

Scheduling notes (from trace analysis):
- each dma_start costs ~620ns of serialized issue time on its sequencer; TRN2 has
  two HWDGE rings (sync + scalar), so input/output DMA issues are split across both.
- output rows for one t are contiguous across u in HBM, so staging C consecutive
  u-iterations in SBUF lets one descriptor cover C*2000 bytes; group sizes taper
  ([2,2,4,8*6,4,2,1,1]) so stores start early and the post-matmul drain is ~1 u.
"""

import sys

import numpy as np

try:
    import concourse.bass as bass
except ImportError:
    sys.path.insert(0, "/opt/trn_rl_repo")
    import concourse.bass as bass

import ml_dtypes

import concourse.mybir as mybir
import concourse.tile as tile
from concourse import bacc
from concourse.bass import ds, ts
from concourse.bass_utils import run_bass_kernel_spmd

N, T, U = 8, 512, 64
E = D = J = 512
V = 500
P = 128
JC = J // P  # 4 chunks of J on partitions
TB = T // P  # 4 blocks of T rows per output tile
F32 = mybir.dt.float32
F16 = mybir.dt.float16
BF16 = mybir.dt.bfloat16

NUM_CORES = 8

# u-iterations per output staging group: start small so stores begin early,
# 4 in steady state (8KB contiguous per descriptor), taper so the final
# drain after the last matmul is a single u (1MB across 4 DMAs).
GROUPS = [2, 2] + [4] * 14 + [2, 1, 1]
assert sum(GROUPS) == U
CMAX = max(GROUPS)


def build_nc() -> bass.Bass:
    nc = bacc.Bacc(
        "TRN2", target_bir_lowering=False, debug=False, num_devices=NUM_CORES
    )
    xT = nc.declare_dram_parameter("xT", [E, T], BF16, isOutput=False)
    dT = nc.declare_dram_parameter("dT", [D, U], BF16, isOutput=False)
    WeT = nc.declare_dram_parameter("WeT", [E, J], BF16, isOutput=False)
    WdT = nc.declare_dram_parameter("WdT", [D, J], BF16, isOutput=False)
    WoT = nc.declare_dram_parameter("WoT", [J, V], BF16, isOutput=False)
    cb = nc.declare_dram_parameter("cb", [J], F32, isOutput=False)
    ob = nc.declare_dram_parameter("ob", [P, V], F32, isOutput=False)
    # fp16 output: halves HBM store traffic (the binding constraint at ~290
    # GB/s/core effective write bandwidth); host upcasts to f32. Logits absmax
    # ~4.6 so fp16 quantization adds only ~5e-4 relative error.
    out = nc.declare_dram_parameter("out", [T, U, V], F16, isOutput=True)

    with tile.TileContext(nc) as tc:
        with (
            tc.tile_pool(name="const", bufs=1) as const_pool,
            tc.tile_pool(name="acts", bufs=3) as act_pool,
            tc.tile_pool(name="obig", bufs=3) as obig_pool,
            tc.tile_pool(name="psenc", bufs=4, space="PSUM") as enc_psum_pool,
            tc.tile_pool(name="psum", bufs=4, space="PSUM") as psum_pool,
        ):
            # ---- persistent SBUF tensors -------------------------------------
            # chunked layouts: [P, chunk, free]
            xT_sb = const_pool.tile([P, E // P, T], BF16, tag="xT")
            dT_sb = const_pool.tile([P, D // P, U], BF16, tag="dT")
            WeT_sb = const_pool.tile([P, E // P, J], BF16, tag="WeT")
            WdT_sb = const_pool.tile([P, D // P, J], BF16, tag="WdT")
            Wo_sb = const_pool.tile([P, JC, V], BF16, tag="WoT")
            cb_sb = const_pool.tile([P, JC], F32, tag="cb")
            ob_sb = const_pool.tile([P, V], F32, tag="ob")
            dec_sb = const_pool.tile([P, JC, U], F32, tag="decT")
            warm_sb = const_pool.tile([P, 64], BF16, tag="warm")

            nc.vector.memset(warm_sb[:], 0.0)

            # ---- input DMAs: coarse transfers, issue split across the two
            # HWDGE rings (sync + scalar); each issue serializes ~620ns on its
            # sequencer, so order = consumption order.
            xT_r = xT.rearrange("(c p) t -> p c t", p=P)
            WeT_r = WeT.rearrange("(c p) j -> p c j", p=P)
            Wo_r = WoT.rearrange("(c p) v -> p c v", p=P)
            # xT on sync, WeT on scalar so each enc chunk's two operands land
            # together via parallel HWDGE rings (gpsimd SWDGE adds ~10us
            # latency -- never put time-critical inputs there). Concurrent
            # transfers round-robin the HBM bandwidth at packet granularity,
            # so anything issued early steals bandwidth from the enc chunks:
            # strict priority order, Wo deferred behind the projections.
            nc.sync.dma_start(xT_sb[:, 0, :], xT_r[:, 0, :])
            nc.scalar.dma_start(WeT_sb[:, 0, :], WeT_r[:, 0, :])
            nc.sync.dma_start(xT_sb[:, 1:4, :], xT_r[:, 1:4, :])
            nc.scalar.dma_start(WeT_sb[:, 1:4, :], WeT_r[:, 1:4, :])
            nc.scalar.dma_start(WdT_sb[:], WdT.rearrange("(c p) j -> p c j", p=P))
            nc.sync.dma_start(dT_sb[:], dT.rearrange("(c p) u -> p c u", p=P))
            nc.sync.dma_start(cb_sb[:], cb.rearrange("(c p) -> p c", p=P))
            nc.sync.dma_start(Wo_sb[:, 0:2, :], Wo_r[:, 0:2, :])
            nc.scalar.dma_start(Wo_sb[:, 2:4, :], Wo_r[:, 2:4, :])
            nc.sync.dma_start(ob_sb[:], ob[:])

            # ---- projections -------------------------------------------------
            # enc stays in PSUM for the whole kernel (4 banks pinned); tanh
            # reads it directly with dec as the per-partition bias.
            ps_enc = [
                enc_psum_pool.tile([P, T], F32, tag="pse", name=f"ps_enc_{jm}")
                for jm in range(JC)
            ]
            ps_dec = [
                psum_pool.tile([P, T], F32, tag="ps", name=f"ps_dec_{jm}")
                for jm in range(JC)
            ]

            # PE warm-up: dummy matmuls while the input DMAs land. Needs >3.4us
            # of sustained PE busy for the HAM clock-gate to lift (1.2 -> 2.4
            # GHz) before the projections, and keeps the PE busy until the
            # first projection operands arrive (~4.5us in: DMA issue+latency).
            # enc's start=True resets the bank afterwards.
            for w in range(66):
                nc.tensor.matmul(
                    ps_enc[0][:64, :64],
                    lhsT=warm_sb[:, :64],
                    rhs=warm_sb[:],
                    start=True,
                    stop=True,
                    skip_group_check=True,
                )

            # ek outer so the first matmuls only need chunk 0 of the DMAs;
            # all enc first (its inputs arrive first), then all dec.
            for ek in range(E // P):
                for jm in range(JC):
                    nc.tensor.matmul(
                        ps_enc[jm][:],
                        lhsT=WeT_sb[:, ek, ts(jm, P)],
                        rhs=xT_sb[:, ek, :],
                        start=(ek == 0),
                        stop=(ek == E // P - 1),
                        skip_group_check=True,
                    )
            for ek in range(E // P):
                for jm in range(JC):
                    nc.tensor.matmul(
                        ps_dec[jm][:, :U],
                        lhsT=WdT_sb[:, ek, ts(jm, P)],
                        rhs=dT_sb[:, ek, :],
                        start=(ek == 0),
                        stop=(ek == E // P - 1),
                        skip_group_check=True,
                    )
            for jm in range(JC):
                nc.scalar.activation(
                    dec_sb[:, jm, :],
                    ps_dec[jm][:, :U],
                    mybir.ActivationFunctionType.Identity,
                    bias=cb_sb[:, jm : jm + 1],
                )

            # ---- main loop over u, grouped for batched output stores ---------
            u = 0
            for g, gs in enumerate(GROUPS):
                ob_t = obig_pool.tile(
                    [P, TB, CMAX, V], F16, tag="obig", name=f"obig_{g}"
                )
                for ui in range(gs):
                    act_t = act_pool.tile(
                        [P, JC, T], BF16, tag="act", name=f"act_{u}"
                    )
                    if u == 0:
                        # first iteration: quarter-granularity tanh in tb-major
                        # order so the first tile's matmuls start ~4x sooner
                        # after the projections finish.
                        for tb in range(TB):
                            for jc in range(JC):
                                nc.scalar.activation(
                                    act_t[:, jc, ts(tb, P)],
                                    ps_enc[jc][:, ts(tb, P)],
                                    mybir.ActivationFunctionType.Tanh,
                                    bias=dec_sb[:, jc, u : u + 1],
                                )
                    else:
                        for jc in range(JC):
                            nc.scalar.activation(
                                act_t[:, jc, :],
                                ps_enc[jc][:],
                                mybir.ActivationFunctionType.Tanh,
                                bias=dec_sb[:, jc, u : u + 1],
                            )
                    for tb in range(TB):
                        ps = psum_pool.tile(
                            [P, T], F32, tag="ps", name=f"ps_{u}_{tb}"
                        )
                        for jc in range(JC):
                            nc.tensor.matmul(
                                ps[:, :V],
                                lhsT=act_t[:, jc, ts(tb, P)],
                                rhs=Wo_sb[:, jc, :],
                                start=(jc == 0),
                                stop=(jc == JC - 1),
                            )
                        nc.vector.tensor_add(
                            ob_t[:, tb, ui, :], ps[:, :V], ob_sb[:]
                        )
                    u += 1
                # store the group: per t row, gs*2000 contiguous bytes in HBM.
                # Mid-stream all stores issue on sync (it is otherwise idle, and
                # a blocking store on scalar would stall the tanh pipeline); the
                # last two groups split across both rings since scalar is done.
                for tb in range(TB):
                    if g >= len(GROUPS) - 2 and tb % 2 == 1:
                        eng = nc.scalar
                    else:
                        eng = nc.sync
                    eng.dma_start(
                        out[ds(tb * P, P), ds(u - gs, gs), :],
                        ob_t[:, tb, 0:gs, :],
                    )

    nc.compile()
    return nc


_CACHED_NC = None


def _get_nc():
    global _CACHED_NC
    if _CACHED_NC is None:
        _CACHED_NC = build_nc()
    return _CACHED_NC


def make_in_maps(
    encoder_out, decoder_out, enc_W, enc_b, dec_W, dec_b, out_W, out_b
) -> list[dict]:
    bf = ml_dtypes.bfloat16
    f32 = np.float32

    def t_bf16(a):  # transpose last two dims, contiguous, bf16
        return np.ascontiguousarray(np.asarray(a, dtype=f32).T).astype(bf)

    WeT = t_bf16(enc_W)  # [E, J]
    WdT = t_bf16(dec_W)  # [D, J]
    WoT = t_bf16(out_W)  # [J, V]
    cb = (np.asarray(enc_b, f32) + np.asarray(dec_b, f32)).astype(f32)  # [J]
    ob = np.ascontiguousarray(
        np.broadcast_to(np.asarray(out_b, f32), (P, V))
    )  # [P, V]

    encoder_out = np.asarray(encoder_out, f32)
    decoder_out = np.asarray(decoder_out, f32)
    in_maps = []
    for i in range(NUM_CORES):
        in_maps.append(
            {
                "xT": t_bf16(encoder_out[i]),  # [E, T]
                "dT": t_bf16(decoder_out[i]),  # [D, U]
                "WeT": WeT,
                "WdT": WdT,
                "WoT": WoT,
                "cb": cb,
                "ob": ob,
            }
        )
    return in_maps


def run(inputs: dict, trace: bool = False):
    """Returns (full_output, BassKernelResults)."""
    nc = _get_nc()
    in_maps = make_in_maps(**inputs)
    res = run_bass_kernel_spmd(
        nc, in_maps, core_ids=list(range(NUM_CORES)), trace=trace
    )
    out = np.stack(
        [res.results[i]["out"] for i in range(NUM_CORES)], axis=0
    )  # (N, T, U, V)
    return np.asarray(out, np.float32), res


def kernel(**inputs) -> np.ndarray:
    out, _ = run(inputs, trace=False)
    return out


# revision 18
# speedup vs baseline: 1.0572x; 1.0005x over previous
"""RNN-T Joiner kernel for Trainium2, data-parallel over batch N across 8 NeuronCores.

Per core (one batch element):
  enc_T[J,T] = enc_W @ x_enc.T          (projection, bf16 matmul, fp32 accum, stays in PSUM)
  dec_T[J,U] = dec_W @ x_dec.T + (enc_b+dec_b)
  act[J,(u,t)] = tanh(enc_T[:,t] + dec_T[:,u])   (ScalarE reads enc from PSUM, dec as bias)
  out[t,u,:] = act.T @ out_W.T + out_b           (PE, act stationary, fp32 PSUM)

Scheduling notes (from trace analysis):
- each dma_start costs ~620ns of serialized issue time on its sequencer; TRN2 has
  two HWDGE rings (sync + scalar), so input/output DMA issues are split across both.
- output rows for one t are contiguous across u in HBM, so staging C consecutive
  u-iterations in SBUF lets one descriptor cover C*2000 bytes; group sizes taper
  ([2,2,4,8*6,4,2,1,1]) so stores start early and the post-matmul drain is ~1 u.
"""

import sys

import numpy as np

try:
    import concourse.bass as bass
except ImportError:
    sys.path.insert(0, "/opt/trn_rl_repo")
    import concourse.bass as bass

import ml_dtypes

import concourse.mybir as mybir
import concourse.tile as tile
from concourse import bacc
from concourse.bass import ds, ts
from concourse.bass_utils import run_bass_kernel_spmd

N, T, U = 8, 512, 64
E = D = J = 512
V = 500
P = 128
JC = J // P  # 4 chunks of J on partitions
TB = T // P  # 4 blocks of T rows per output tile
F32 = mybir.dt.float32
F16 = mybir.dt.float16
BF16 = mybir.dt.bfloat16

NUM_CORES = 8

# u-iterations per output staging group: start small so stores begin early,
# 4 in steady state (8KB contiguous per descriptor), taper so the final
# drain after the last matmul is a single u (1MB across 4 DMAs).
GROUPS = [2, 2] + [4] * 14 + [2, 1, 1]
assert sum(GROUPS) == U
CMAX = max(GROUPS)


def build_nc() -> bass.Bass:
    nc = bacc.Bacc(
        "TRN2", target_bir_lowering=False, debug=False, num_devices=NUM_CORES
    )
    xT = nc.declare_dram_parameter("xT", [E, T], F16, isOutput=False)
    dT = nc.declare_dram_parameter("dT", [D, U], F16, isOutput=False)
    WeT = nc.declare_dram_parameter("WeT", [E, J], F16, isOutput=False)
    WdT = nc.declare_dram_parameter("WdT", [D, J], F16, isOutput=False)
    WoT = nc.declare_dram_parameter("WoT", [J, V], F16, isOutput=False)
    cb = nc.declare_dram_parameter("cb", [J], F32, isOutput=False)
    ob = nc.declare_dram_parameter("ob", [P, V], F32, isOutput=False)
    # fp16 output: halves HBM store traffic (the binding constraint at ~290
    # GB/s/core effective write bandwidth); host upcasts to f32. Logits absmax
    # ~4.6 so fp16 quantization adds only ~5e-4 relative error.
    out = nc.declare_dram_parameter("out", [T, U, V], F16, isOutput=True)

    with tile.TileContext(nc) as tc:
        with (
            tc.tile_pool(name="const", bufs=1) as const_pool,
            tc.tile_pool(name="acts", bufs=3) as act_pool,
            tc.tile_pool(name="obig", bufs=3) as obig_pool,
            tc.tile_pool(name="psenc", bufs=4, space="PSUM") as enc_psum_pool,
            tc.tile_pool(name="psum", bufs=4, space="PSUM") as psum_pool,
        ):
            # ---- persistent SBUF tensors -------------------------------------
            # chunked layouts: [P, chunk, free]
            xT_sb = const_pool.tile([P, E // P, T], F16, tag="xT")
            dT_sb = const_pool.tile([P, D // P, U], F16, tag="dT")
            WeT_sb = const_pool.tile([P, E // P, J], F16, tag="WeT")
            WdT_sb = const_pool.tile([P, D // P, J], F16, tag="WdT")
            Wo_sb = const_pool.tile([P, JC, V], F16, tag="WoT")
            cb_sb = const_pool.tile([P, JC], F32, tag="cb")
            ob_sb = const_pool.tile([P, V], F32, tag="ob")
            dec_sb = const_pool.tile([P, JC, U], F32, tag="decT")
            warm_sb = const_pool.tile([P, 64], F16, tag="warm")

            nc.vector.memset(warm_sb[:], 0.0)

            # ---- input DMAs: coarse transfers, issue split across the two
            # HWDGE rings (sync + scalar); each issue serializes ~620ns on its
            # sequencer, so order = consumption order.
            xT_r = xT.rearrange("(c p) t -> p c t", p=P)
            WeT_r = WeT.rearrange("(c p) j -> p c j", p=P)
            Wo_r = WoT.rearrange("(c p) v -> p c v", p=P)
            # xT on sync, WeT on scalar so each enc chunk's two operands land
            # together via parallel HWDGE rings (gpsimd SWDGE adds ~10us
            # latency -- never put time-critical inputs there). Concurrent
            # transfers round-robin the HBM bandwidth at packet granularity,
            # so anything issued early steals bandwidth from the enc chunks:
            # strict priority order, Wo deferred behind the projections.
            nc.sync.dma_start(xT_sb[:, 0, :], xT_r[:, 0, :])
            nc.scalar.dma_start(WeT_sb[:, 0, :], WeT_r[:, 0, :])
            nc.sync.dma_start(xT_sb[:, 1:4, :], xT_r[:, 1:4, :])
            nc.scalar.dma_start(WeT_sb[:, 1:4, :], WeT_r[:, 1:4, :])
            nc.scalar.dma_start(WdT_sb[:], WdT.rearrange("(c p) j -> p c j", p=P))
            nc.sync.dma_start(dT_sb[:], dT.rearrange("(c p) u -> p c u", p=P))
            nc.sync.dma_start(cb_sb[:], cb.rearrange("(c p) -> p c", p=P))
            nc.sync.dma_start(Wo_sb[:, 0:2, :], Wo_r[:, 0:2, :])
            nc.scalar.dma_start(Wo_sb[:, 2:4, :], Wo_r[:, 2:4, :])
            nc.sync.dma_start(ob_sb[:], ob[:])

            # ---- projections -------------------------------------------------
            # enc stays in PSUM for the whole kernel (4 banks pinned); tanh
            # reads it directly with dec as the per-partition bias.
            ps_enc = [
                enc_psum_pool.tile([P, T], F32, tag="pse", name=f"ps_enc_{jm}")
                for jm in range(JC)
            ]
            ps_dec = [
                psum_pool.tile([P, T], F32, tag="ps", name=f"ps_dec_{jm}")
                for jm in range(JC)
            ]

            # PE warm-up: dummy matmuls while the input DMAs land. Needs >3.4us
            # of sustained PE busy for the HAM clock-gate to lift (1.2 -> 2.4
            # GHz) before the projections, and keeps the PE busy until the
            # first projection operands arrive (~4.5us in: DMA issue+latency).
            # enc's start=True resets the bank afterwards.
            for w in range(66):
                nc.tensor.matmul(
                    ps_enc[0][:64, :64],
                    lhsT=warm_sb[:, :64],
                    rhs=warm_sb[:],
                    start=True,
                    stop=True,
                    skip_group_check=True,
                )

            # ek outer so the first matmuls only need chunk 0 of the DMAs;
            # all enc first (its inputs arrive first), then all dec.
            for ek in range(E // P):
                for jm in range(JC):
                    nc.tensor.matmul(
                        ps_enc[jm][:],
                        lhsT=WeT_sb[:, ek, ts(jm, P)],
                        rhs=xT_sb[:, ek, :],
                        start=(ek == 0),
                        stop=(ek == E // P - 1),
                        skip_group_check=True,
                    )
            for jm in range(JC):
                for ek in range(E // P):
                    nc.tensor.matmul(
                        ps_dec[jm][:, :U],
                        lhsT=WdT_sb[:, ek, ts(jm, P)],
                        rhs=dT_sb[:, ek, :],
                        start=(ek == 0),
                        stop=(ek == E // P - 1),
                        skip_group_check=True,
                    )
                nc.scalar.activation(
                    dec_sb[:, jm, :],
                    ps_dec[jm][:, :U],
                    mybir.ActivationFunctionType.Identity,
                    bias=cb_sb[:, jm : jm + 1],
                )

            # ---- main loop over u, grouped for batched output stores ---------
            u = 0
            for g, gs in enumerate(GROUPS):
                ob_t = obig_pool.tile(
                    [P, TB, CMAX, V], F16, tag="obig", name=f"obig_{g}"
                )
                for ui in range(gs):
                    act_t = act_pool.tile(
                        [P, JC, T], F16, tag="act", name=f"act_{u}"
                    )
                    if u == 0:
                        # first iteration: quarter-granularity tanh in tb-major
                        # order so the first tile's matmuls start ~4x sooner
                        # after the projections finish.
                        for tb in range(TB):
                            for jc in range(JC):
                                nc.scalar.activation(
                                    act_t[:, jc, ts(tb, P)],
                                    ps_enc[jc][:, ts(tb, P)],
                                    mybir.ActivationFunctionType.Tanh,
                                    bias=dec_sb[:, jc, u : u + 1],
                                )
                    else:
                        for jc in range(JC):
                            nc.scalar.activation(
                                act_t[:, jc, :],
                                ps_enc[jc][:],
                                mybir.ActivationFunctionType.Tanh,
                                bias=dec_sb[:, jc, u : u + 1],
                            )
                    for tb in range(TB):
                        ps = psum_pool.tile(
                            [P, T], F32, tag="ps", name=f"ps_{u}_{tb}"
                        )
                        for jc in range(JC):
                            nc.tensor.matmul(
                                ps[:, :V],
                                lhsT=act_t[:, jc, ts(tb, P)],
                                rhs=Wo_sb[:, jc, :],
                                start=(jc == 0),
                                stop=(jc == JC - 1),
                            )
                        nc.vector.tensor_add(
                            ob_t[:, tb, ui, :], ps[:, :V], ob_sb[:]
                        )
                    u += 1
                # store the group: per t row, gs*2000 contiguous bytes in HBM.
                # Mid-stream all stores issue on sync (it is otherwise idle, and
                # a blocking store on scalar would stall the tanh pipeline); the
                # last two groups split across both rings since scalar is done.
                for tb in range(TB):
                    if g >= len(GROUPS) - 2 and tb % 2 == 1:
                        eng = nc.scalar
                    else:
                        eng = nc.sync
                    eng.dma_start(
                        out[ds(tb * P, P), ds(u - gs, gs), :],
                        ob_t[:, tb, 0:gs, :],
                    )

    nc.compile()
    return nc


_CACHED_NC = None


def _get_nc():
    global _CACHED_NC
    if _CACHED_NC is None:
        _CACHED_NC = build_nc()
    return _CACHED_NC


def make_in_maps(
    encoder_out, decoder_out, enc_W, enc_b, dec_W, dec_b, out_W, out_b
) -> list[dict]:
    f16 = np.float16
    f32 = np.float32

    def t_f16(a):  # transpose last two dims, contiguous, fp16
        return np.ascontiguousarray(np.asarray(a, dtype=f32).T).astype(f16)

    WeT = t_f16(enc_W)  # [E, J]
    WdT = t_f16(dec_W)  # [D, J]
    WoT = t_f16(out_W)  # [J, V]
    cb = (np.asarray(enc_b, f32) + np.asarray(dec_b, f32)).astype(f32)  # [J]
    ob = np.ascontiguousarray(
        np.broadcast_to(np.asarray(out_b, f32), (P, V))
    )  # [P, V]

    encoder_out = np.asarray(encoder_out, f32)
    decoder_out = np.asarray(decoder_out, f32)
    in_maps = []
    for i in range(NUM_CORES):
        in_maps.append(
            {
                "xT": t_f16(encoder_out[i]),  # [E, T]
                "dT": t_f16(decoder_out[i]),  # [D, U]
                "WeT": WeT,
                "WdT": WdT,
                "WoT": WoT,
                "cb": cb,
                "ob": ob,
            }
        )
    return in_maps


def run(inputs: dict, trace: bool = False):
    """Returns (full_output, BassKernelResults)."""
    nc = _get_nc()
    in_maps = make_in_maps(**inputs)
    res = run_bass_kernel_spmd(
        nc, in_maps, core_ids=list(range(NUM_CORES)), trace=trace
    )
    out = np.stack(
        [res.results[i]["out"] for i in range(NUM_CORES)], axis=0
    )  # (N, T, U, V)
    return np.asarray(out, np.float32), res


def kernel(**inputs) -> np.ndarray:
    out, _ = run(inputs, trace=False)
    return out
